# revision 25
# baseline (speedup 1.0000x reference)
"""Multi-head attention Trainium2 Bass kernel.

Problem: x:(4,512,1024), Wq/Wk/Wv/Wo:(512,512), H=8 heads, d=64.
  q = Wq@x ; k = Wk@x ; v = Wv@x  (per batch, 1x1 conv == channel matmul)
  per head: S[i,j] = q[:,i].k[:,j] ; attn = softmax_j(S) ; y = attn @ v
  out = Wo @ y
Sharding: 8 cores = (batch b, head-half g).  Core (b,g) handles batch b,
local heads g*4..g*4+3 and computes the partial output
out_p = Wo[:, g*256:(g+1)*256] @ y_g which the host sums pairwise.

Layout: scores are computed TRANSPOSED (S^T[j,i] = k^T q) so no PE
transposes are needed anywhere; softmax runs without max subtraction
(scores max ~52 < 88 overflow limit); the PV matmul's lhsT carries an
extra ones column so the softmax denominator falls out of the same
matmul; the rowsum row is replicated across partitions by a K=1
ones-row matmul and inverted with a fast approximate reciprocal.

Schedule: the kernel is ACT-bound in the middle (32 exp calls ~37us),
so everything is ordered to start that stream as early as possible and
keep it dense: input DMA descriptors split across two engine queues
(x on sync, w on gpsimd), a short PE warm-up bridges the DMA window so
HAM unthrottles once and stays warm, q/k projections for the first
i-half run kc-major into 4 concurrent PSUM accumulators the moment
each x chunk lands, and the remaining projections (v tiles, nn=1
halves, the i-half-0 output projection) are drip-fed as fillers into
the attention blocks' PE/DVE slack.  SBUF tiles are split per consumer
(q/k per (m, nn-half), y per (head, i-half)) because Tile tracks
dependencies per tile.
"""

import numpy as np

import concourse.bass as bass
import concourse.tile as tile
from concourse import bacc
from concourse import mybir
from concourse.bass_utils import run_bass_kernel_spmd

F32 = mybir.dt.float32
F32R = mybir.dt.float32r
F16 = mybir.dt.float16
U32 = mybir.dt.uint32

P = 128
C = 512          # channels
NSEQ = 1024      # sequence length
D = 64           # head dim
HL = 4           # local heads per core
KC = C // P      # 4 contraction tiles over channels
J = NSEQ // P    # 8 key tiles
EXP = mybir.ActivationFunctionType.Exp

_NC_CACHE = {}


def build_nc():
    nc = bacc.Bacc("TRN2")

    x = nc.dram_tensor("x", [C, NSEQ], F16, kind="ExternalInput")
    wqkv = nc.dram_tensor("wqkv_t", [C, 3, 2 * P], F16, kind="ExternalInput")
    wo = nc.dram_tensor("wo_t", [D, HL, C], F16, kind="ExternalInput")
    out = nc.dram_tensor("out_p", [C, NSEQ], F16, kind="ExternalOutput")

    with tile.TileContext(nc) as tc:
        with (
            tc.tile_pool(name="consts", bufs=1) as consts,
            tc.tile_pool(name="epool", bufs=6) as epool,
            tc.tile_pool(name="ypool", bufs=6) as ypool,
            tc.tile_pool(name="rpool", bufs=4) as rpool,
            tc.tile_pool(name="opool", bufs=4) as opool,
            tc.tile_pool(name="pp", bufs=2, space="PSUM") as pp,
        ):
            x_t = x.rearrange("(kc p) n -> p kc n", p=P)
            w_t = wqkv.rearrange("(kc p) w m -> p kc w m", p=P)

            # ---- PE warm-up: dependency-free matmuls bridge the input
            # DMA window so HAM sees sustained activity before the real
            # projections; sized to end roughly when the first chunks
            # land (~2us).
            warm = consts.tile([P, P + 512], F32R)
            nc.vector.memset(warm.bitcast(U32), 0)
            for wi in range(6):
                pw = pp.tile([P, 512], F32, tag="po", name="pw", bufs=2)
                nc.tensor.matmul(pw, lhsT=warm[:, 0:P], rhs=warm[:, P:],
                                 start=True, stop=True)

            # ---- constants: the vt ones columns and denominator ones
            # row.  f32r storage is plain f32 bits and 1.0 is exact, so
            # the ones are direct bit-pattern memsets; one tiny ACT exp
            # stays as the deliberate table-set load trigger.
            vt_sb = [
                consts.tile([P, HL, D + 1], F32R, tag=f"vt{j}", name=f"vt{j}")
                for j in range(J)
            ]
            for j in range(J):
                nc.vector.memset(
                    vt_sb[j][:, :, D:D + 1].bitcast(U32), 0x3F800000)
            ones64 = consts.tile([P, D], F32R)
            nc.vector.memset(ones64.bitcast(U32), 0)
            nc.scalar.activation(out=ones64[D:D + 1, :], in_=ones64[D:D + 1, :],
                                 func=EXP, bias=0.0, scale=0.0)

            # ---- input DMA: x (kc, nn-half) chunks on the sync queue,
            # weights on the gpsimd queue so descriptor issue overlaps.
            # nn=0 chunks first: they gate the first attention block.
            w3_sb = []
            for kc in range(KC):
                t = consts.tile([P, 3, 2 * P], F16, tag=f"w{kc}")
                nc.scalar.dma_start(t, w_t[:, kc])
                w3_sb.append(t)
            x_sb = [[None, None] for _ in range(KC)]
            for kc in range(KC):
                t = consts.tile([P, 512], F16, tag=f"x{kc}0")
                nc.sync.dma_start(t, x_t[:, kc, 0:512])
                x_sb[kc][0] = t
            for kc in range(KC):
                t = consts.tile([P, 512], F16, tag=f"x{kc}1")
                nc.scalar.dma_start(t, x_t[:, kc, 512:1024])
                x_sb[kc][1] = t
            wot_sb = consts.tile([D, HL, C], F16)
            nc.scalar.dma_start(wot_sb, wo[:, :, :])
            wq_sb = [t[:, 0, :] for t in w3_sb]
            wk_sb = [t[:, 1, :] for t in w3_sb]
            wv_sb = [t[:, 2, :] for t in w3_sb]

            # ---- q/k projections: per (m, nn) tiles [128, 512].
            # nn=0 groups run kc-major into 4 concurrent accumulators so
            # the PE chases the x DMA chunk-by-chunk; casts start the
            # moment a group's kc=3 lands.  The first pair's k cast goes
            # on ACT (idle until the first exp) so both of that pair's
            # casts finish in parallel.
            q_sb = [[consts.tile([P, 512], F16, tag=f"q{m}{nn}",
                                 name=f"q{m}{nn}")
                     for nn in range(2)] for m in range(2)]
            k_sb = [[consts.tile([P, 512], F16, tag=f"k{m}{nn}",
                                 name=f"k{m}{nn}")
                     for nn in range(2)] for m in range(2)]

            def qk_group(w_sb, m, nn, tag):
                acc = pp.tile([P, 512], F32, tag=tag, name=tag,
                              bufs=2 if tag == "po" else 1)
                return acc, lambda kc: nc.tensor.matmul(
                    acc,
                    lhsT=w_sb[kc][:, m * P:(m + 1) * P],
                    rhs=x_sb[kc][nn],
                    start=(kc == 0),
                    stop=(kc == KC - 1),
                )

            g_specs = [("k", 0, "po"), ("q", 0, "po"),
                       ("k", 1, "py0"), ("q", 1, "py1")]
            accs = {}
            mms = {}
            for wname, m, tag in g_specs:
                acc, mm = qk_group(wk_sb if wname == "k" else wq_sb, m, 0, tag)
                accs[(wname, m)] = acc
                mms[(wname, m)] = mm
            for kc in range(KC):
                for wname, m, _ in g_specs:
                    mms[(wname, m)](kc)
            nc.scalar.copy(out=k_sb[0][0], in_=accs[("k", 0)])
            nc.vector.tensor_copy(out=q_sb[0][0], in_=accs[("q", 0)])

            # ---- v projection helper: vt[j] = x_j^T @ Wv (+ ones col).
            def v_group(j, tag):
                nn, jj = j // 4, j % 4
                psv = pp.tile([P, 512], F32, tag=tag, name=tag,
                              bufs=2 if tag == "po" else 1)
                for kc in range(KC):
                    nc.tensor.matmul(
                        psv[:, 0:2 * P],
                        lhsT=x_sb[kc][nn][:, jj * P:(jj + 1) * P],
                        rhs=wv_sb[kc],
                        start=(kc == 0),
                        stop=(kc == KC - 1),
                    )
                nc.vector.tensor_copy(
                    out=vt_sb[j][:, :, 0:D],
                    in_=psv[:, 0:2 * P].rearrange("p (h d) -> p h d", h=HL),
                )

            # pair-1 casts gate the first block's PV slots
            nc.vector.tensor_copy(out=k_sb[1][0], in_=accs[("k", 1)])
            nc.vector.tensor_copy(out=q_sb[1][0], in_=accs[("q", 1)])

            def qk_nn1_mms(wname, m):
                acc, mm = qk_group(wk_sb if wname == "k" else wq_sb, m, 1, "po")
                accs[(wname, m, 1)] = acc
                for kc in range(KC):
                    mm(kc)

            def qk_nn1_cast(wname, m):
                dst = (k_sb if wname == "k" else q_sb)[m][1]
                nc.vector.tensor_copy(out=dst, in_=accs[(wname, m, 1)])

            # ---- filler work dripped into the attention blocks:
            # (block, j) -> list of closures.  Each slot is <=1us of PE
            # so it never starves the score->exp cadence.  Placement is
            # tuned against the "po" PSUM slot rotation: at each block
            # boundary the drain's two pr matmuls must find slots freed
            # by casts, never by work that hasn't run yet.
            fillers = {
                (0, 0): [lambda: v_group(0, "po")],
                (0, 1): [lambda: v_group(1, "po"),
                         lambda: qk_nn1_mms("k", 0)],
                (0, 2): [lambda: qk_nn1_cast("k", 0),
                         lambda: v_group(2, "po")],
                (0, 3): [lambda: v_group(3, "po")],
                (0, 4): [lambda: v_group(4, "po")],
                (0, 5): [lambda: v_group(5, "po")],
                (0, 6): [lambda: v_group(6, "po")],
                (0, 7): [lambda: v_group(7, "po")],
                (1, 1): [lambda: qk_nn1_mms("k", 1)],
                (1, 2): [lambda: qk_nn1_cast("k", 1),
                         lambda: qk_nn1_mms("q", 1)],
                (1, 3): [lambda: qk_nn1_cast("q", 1)],
                (1, 4): [lambda: qk_nn1_mms("q", 0)],
                (1, 5): [lambda: qk_nn1_cast("q", 0)],
            }

            # y per (head, ihalf) tiles so the ihalf-0 output projection
            # doesn't wait on ihalf-1 drains (per-tile dep tracking).
            y_sb = [[consts.tile([D, 512], F16, tag=f"y{h}{i}",
                                 name=f"y{h}{i}")
                     for i in range(2)] for h in range(HL)]
            out_t = out.rearrange("(m p) n -> p m n", p=P)

            def drain_copies(py, last):
                # copy the 65-row PV accumulators out (frees the PV
                # slots for the next block).  On the final block the
                # lane-1 copy goes to ACT, which is done with exps.
                yus = []
                for lane in range(2):
                    yu = ypool.tile([D + 1, 512], F32R, tag="yu", name="yu")
                    if last and lane == 1:
                        nc.scalar.copy(out=yu, in_=py[lane][0:D + 1, :])
                    else:
                        nc.vector.tensor_copy(out=yu, in_=py[lane][0:D + 1, :])
                    yus.append(yu)
                return yus

            def drain_finish(yus, lane, pair, ihalf):
                # softmax denominator: replicate row 64 across
                # partitions with a K=1 matmul, reciprocal, scale.
                h = 2 * pair + lane
                pr = pp.tile([D, 512], F32, tag="po", name="pr", bufs=2)
                nc.tensor.matmul(
                    pr, lhsT=ones64[D:D + 1, :], rhs=yus[lane][D:D + 1, :],
                    start=True, stop=True,
                )
                rr = rpool.tile([D, 512], F32, tag="rr", name="rr")
                nc.vector.reciprocal_approx_fast(out=rr, in_=pr)
                nc.vector.tensor_tensor(
                    out=y_sb[h][ihalf], in0=yus[lane][0:D, :], in1=rr,
                    op=mybir.AluOpType.mult,
                )

            def outproj_m(ihalf, horder, m, tag="po", dma=None,
                          act_cast=False):
                po = pp.tile([P, 512], F32, tag=tag, name="po",
                             bufs=2 if tag == "po" else 1)
                for hi, h in enumerate(horder):
                    nc.tensor.matmul(
                        po,
                        lhsT=wot_sb[:, h, m * P:(m + 1) * P],
                        rhs=y_sb[h][ihalf],
                        start=(hi == 0),
                        stop=(hi == HL - 1),
                    )
                ot = opool.tile([P, 512], F16, tag="ot")
                if act_cast:
                    nc.scalar.copy(out=ot, in_=po)
                else:
                    nc.vector.tensor_copy(out=ot, in_=po)
                (dma or nc.sync).dma_start(
                    out=out_t[:, m, ihalf * 512:(ihalf + 1) * 512], in_=ot)

            # ihalf-0 output projection drips into block 2; pair-0 heads
            # (drained a block earlier) lead the accumulation so each
            # m-group starts before pair-1's drain finishes.
            fillers[(2, 2)] = [lambda: outproj_m(0, (0, 1, 2, 3), 0)]
            fillers[(2, 3)] = [lambda: outproj_m(0, (0, 1, 2, 3), 1)]
            fillers[(2, 5)] = [lambda: outproj_m(0, (0, 1, 2, 3), 2)]
            fillers[(2, 6)] = [lambda: outproj_m(0, (0, 1, 2, 3), 3)]

            # ---- attention blocks.  Scores for both lanes land in one
            # 2-bank PSUM tile (the K=64 matmuls auto-row-tile via base
            # partitions 0/64 and run concurrently) so a single
            # 1024-wide exp serves the head pair.  PV accumulates into
            # py0/py1 with the ones column producing the denominator.
            # j-steps run in batches of two so the PE stays in one
            # tiling mode longer (K=64 score pairs, then K=128 PV +
            # filler matmuls) -- mode changes flush the weight-load
            # pipelining, costing ~0.3us per switch.
            blocks = [(0, 0), (0, 1), (1, 1), (1, 0)]
            pending_tail = []
            for bi, (ihalf, pair) in enumerate(blocks):
                py = [
                    pp.tile([P, 512], F32, tag="py0", name="py0", bufs=1),
                    pp.tile([P, 512], F32, tag="py1", name="py1", bufs=1),
                ]
                es = []

                def pv(jp, py=py, pair=pair, es=es):
                    for lane in range(2):
                        nc.tensor.matmul(
                            py[lane][0:D + 1, :],
                            lhsT=vt_sb[jp][:, 2 * pair + lane, :],
                            rhs=es[jp][:, lane * 512:(lane + 1) * 512],
                            start=(jp == 0), stop=(jp == J - 1),
                        )

                for jb in range(0, J, 2):
                    for j in (jb, jb + 1):
                        nn, jj = j // 4, j % 4
                        ps = pp.tile([P, 1024], F32, tag="s", name="s", bufs=2)
                        for lane in range(2):
                            hp = lane * D
                            nc.tensor.matmul(
                                ps[:, lane * 512:(lane + 1) * 512],
                                lhsT=k_sb[pair][nn][hp:hp + D,
                                                    jj * P:(jj + 1) * P],
                                rhs=q_sb[pair][ihalf][hp:hp + D, :],
                                start=True, stop=True,
                            )
                        e = epool.tile([P, 1024], F32R, tag="e", name="e")
                        nc.scalar.activation(out=e, in_=ps, func=EXP)
                        es.append(e)
                    if jb == 0:
                        # previous block's last PVs + drain copies run
                        # here, AFTER this block's first scores batch is
                        # in the PE FIFO -- they wait on the previous
                        # exp j7, and queuing them first idled the PE
                        # ~1us at every block boundary.
                        for fn in pending_tail:
                            fn()
                        pending_tail = []
                    else:
                        pv(jb - 2)
                        pv(jb - 1)
                    for j in (jb, jb + 1):
                        for fn in fillers.get((bi, j), ()):
                            fn()
                last = bi == len(blocks) - 1
                if last:
                    pv(J - 2)
                    pk = pp.tile([P, 512], F32, tag="s", name="pk", bufs=2)
                    nc.tensor.matmul(pk, lhsT=warm[:, 0:P], rhs=warm[:, P:],
                                     start=True, stop=True)
                    pv(J - 1)
                    for wi in range(5):
                        pk = pp.tile([P, 512], F32, tag="s", name="pk", bufs=2)
                        nc.tensor.matmul(pk, lhsT=warm[:, 0:P],
                                         rhs=warm[:, P:], start=True,
                                         stop=True)
                    yus = drain_copies(py, last)
                    drain_finish(yus, 0, pair, ihalf)
                    drain_finish(yus, 1, pair, ihalf)
                else:
                    def tail_fn(py=py, pair=pair, ihalf=ihalf, pv=pv,
                                nbi=bi + 1):
                        pv(J - 2)
                        pv(J - 1)
                        yus = drain_copies(py, False)
                        for lane in (0, 1):
                            fillers.setdefault((nbi, lane), []).insert(
                                0, (lambda yus=yus, lane=lane, pair=pair,
                                    ihalf=ihalf:
                                    drain_finish(yus, lane, pair, ihalf)))
                    pending_tail.append(tail_fn)

            # ihalf-1 output projection: pair-1 heads (drained during
            # the final block) lead, so every m-group starts its first
            # two matmuls right away; four concurrent accumulators (po
            # x2 plus the just-freed PV slots) keep the PE busy through
            # the final drain; output DMA rides both HWDGE rings.
            outproj_m(1, (2, 3, 0, 1), 0, tag="po", dma=nc.sync)
            outproj_m(1, (2, 3, 0, 1), 1, tag="po", dma=nc.scalar,
                      act_cast=True)
            outproj_m(1, (3, 2, 1, 0), 2, tag="py0", dma=nc.sync)
            outproj_m(1, (3, 2, 1, 0), 3, tag="py1", dma=nc.scalar,
                      act_cast=True)

    nc.compile()
    return nc


def get_nc():
    if "nc" not in _NC_CACHE:
        _NC_CACHE["nc"] = build_nc()
    return _NC_CACHE["nc"]


def make_in_maps(x, Wq, Wk, Wv, Wo):
    in_maps = []
    for core in range(8):
        b, g = core // 2, core % 2
        sl = slice(g * 256, (g + 1) * 256)
        wqkv = np.stack(
            [Wq[sl, :].T, Wk[sl, :].T, Wv[sl, :].T], axis=1
        )  # (512, 3, 256)
        in_maps.append({
            "x": np.ascontiguousarray(x[b]).astype(np.float16),
            "wqkv_t": np.ascontiguousarray(wqkv).astype(np.float16),
            # [d, h, o] so lhsT slices are contiguous per head
            "wo_t": np.ascontiguousarray(
                Wo[:, sl].reshape(C, HL, D).transpose(2, 1, 0)
            ).astype(np.float16),
        })
    return in_maps


LAST_RESULTS = {}


def kernel(x, Wq, Wk, Wv, Wo, _trace=False):
    x = np.asarray(x, dtype=np.float32)
    Wq = np.asarray(Wq, dtype=np.float32)
    Wk = np.asarray(Wk, dtype=np.float32)
    Wv = np.asarray(Wv, dtype=np.float32)
    Wo = np.asarray(Wo, dtype=np.float32)

    nc = get_nc()
    in_maps = make_in_maps(x, Wq, Wk, Wv, Wo)
    res = run_bass_kernel_spmd(
        nc, in_maps, core_ids=list(range(8)), trace=_trace
    )
    LAST_RESULTS["res"] = res
    parts = [np.asarray(r["out_p"], dtype=np.float32) for r in res.results]
    out = np.stack([parts[2 * b] + parts[2 * b + 1] for b in range(4)])
    return out
